# revision 22
# baseline (speedup 1.0000x reference)
"""Trainium2 Bass kernel for nn_BitwiseWavenetBlock (10-layer gated WaveNet block).

Strategy: data-parallel over batch (16 batches -> 8 cores x 2 sequential).
Each core runs the full 10-layer block on [128, 16384] activations resident in
SBUF. Dilated k=2 convs are pairs of PSUM-accumulated 128x128 matmuls against
shifted windows of an fp16 activation buffer with zeroed margins. Weight
gating (W * sigmoid(G)), per-channel scale s and biases are folded on the host.

Engine split per 1024-col chunk pair: PE 12 matmuls (N=512, weight-major so
each stationary matrix streams both chunks); ACT bias-add + fp16 cast of the
filter/gate PSUM halves (512-wide, double-buffered banks); DVE gating multiply
plus fused (psum + bias) + master accumulations (1024-wide); on alternating
pairs the residual update is ACT-assisted to balance ACT/DVE. The residual
master lives in fp16 directly in the conv input buffer xb (updated in place);
the skip master accumulates in fp32. Each pair's skip/residual 1x1 matmuls and
master updates are deferred two pairs so the in-place xb write never races the
backward conv tap and the PE never waits on the gating chain.
"""

import sys

sys.path.insert(0, "/opt/trn_rl_repo")

import numpy as np

import concourse.bass as bass
import concourse.bacc as bacc
import concourse.mybir as mybir
import concourse.tile as tile
from concourse.bass_utils import run_bass_kernel_spmd

F32 = mybir.dt.float32
F16 = mybir.dt.float16
AF = mybir.ActivationFunctionType
ALU = mybir.AluOpType

N_CORES = 8
LAYERS = 10
C = 128          # channels (= partitions)
L = 16384        # sequence length
B = 16           # total batch
BPC = B // N_CORES  # batches per core (sequential)
CW = 512         # chunk width (1 PSUM bank)
MG = 256         # xb margin (>= max shift 2^8)
NH = CW // 512   # matmul halves per chunk


def _build_nc(bpc=BPC, l_len=L, layers=LAYERS, psum_bufs=2, reps=1, ew_width=None,
              mm_n=512, ew_act=None, ew_dve=None, ew_pool=None):
    if mm_n > 512:
        bass.BassTensorEngine.MAX_MOVING_FREE_DIM_SIZE = mm_n
    # ew_*: attribution-only knobs - slice that engine's ops to this many
    # columns (keeps op count/deps, removes streaming work). None = full.
    ew_act = ew_act if ew_act is not None else ew_width
    ew_dve = ew_dve if ew_dve is not None else ew_width
    ew_pool = ew_pool if ew_pool is not None else ew_width
    def _w(ap, w):
        return ap if w is None else ap[:, :w]
    def Wa(ap):
        return _w(ap, ew_act)
    def Wd(ap):
        return _w(ap, ew_dve)
    def Wp(ap):
        return _w(ap, ew_pool)
    nch = l_len // CW
    nc = bacc.Bacc(None)
    x_in = nc.declare_dram_parameter("x", [bpc, C, l_len], F32, isOutput=False)
    wts_in = nc.declare_dram_parameter("wts", [C, layers * 6 * C], F16, isOutput=False)
    bias_in = nc.declare_dram_parameter("biases", [C, layers * 4], F32, isOutput=False)
    resid_out = nc.declare_dram_parameter("resid", [bpc, C, l_len], F32, isOutput=True)
    skip_out = nc.declare_dram_parameter("skip", [bpc, C, l_len], F32, isOutput=True)

    with tile.TileContext(nc) as tc:
        with (
            tc.tile_pool(name="constp", bufs=1) as constp,
            tc.tile_pool(name="masterp", bufs=1) as masterp,
            tc.tile_pool(name="workp", bufs=4) as workp,
            tc.tile_pool(name="psump", bufs=1, space="PSUM") as psump,
        ):
            wts = constp.tile([C, layers * 6 * C], F16)
            nc.sync.dma_start(wts[:], wts_in[:])
            biases = constp.tile([C, layers * 4], F32)
            nc.sync.dma_start(biases[:], bias_in[:])

            skip_m = masterp.tile([C, l_len], F32)
            xb = masterp.tile([C, l_len + 2 * MG], F16)
            nc.vector.memset(xb[:], 0.0)

            def wmat(l, j):
                return wts[:, (l * 6 + j) * C : (l * 6 + j + 1) * C]

            def bvec(l, j):
                return biases[:, l * 4 + j : l * 4 + j + 1]

            def xbc(c):
                return xb[:, MG + c * CW : MG + (c + 1) * CW]

            for b in range(bpc * reps):
                b = b % bpc
                for c in range(nch):
                    stg_in = workp.tile([C, CW], F32, tag="stg", name="stg_in")
                    nc.sync.dma_start(stg_in[:], x_in[b, :, c * CW : (c + 1) * CW])
                    nc.vector.tensor_copy(Wd(xbc(c)), Wd(stg_in[:]))
                for l in range(layers):
                    s0 = 1 if l == 0 else 2 ** (l - 1)
                    s1 = 0 if l == 0 else 2 ** (l - 1)
                    h_pair = []
                    pend = []

                    def sr_phase(c0, h_a, h_b, h_full, l=l, b=b):
                        # skip/resid 1x1 convs + master updates over a PAIR of
                        # chunks: 1024-wide PSUM tiles keep the DVE ops wide
                        s_ps = psump.tile([C, 2 * CW], F32, tag="s", name="s_ps", bufs=1)
                        r_ps = psump.tile([C, 2 * CW], F32, tag="r", name="r_ps", bufs=1)
                        if mm_n == 2 * CW:
                            nc.tensor.matmul(s_ps[:], wmat(l, 4), h_full[:], start=True, stop=True)
                            nc.tensor.matmul(r_ps[:], wmat(l, 5), h_full[:], start=True, stop=True)
                        else:
                            for i, h_t in ((0, h_a), (1, h_b)):
                                hs = slice(i * CW, (i + 1) * CW)
                                nc.tensor.matmul(s_ps[:, hs], wmat(l, 4), h_t[:], start=True, stop=True)
                            for i, h_t in ((0, h_a), (1, h_b)):
                                hs = slice(i * CW, (i + 1) * CW)
                                nc.tensor.matmul(r_ps[:, hs], wmat(l, 5), h_t[:], start=True, stop=True)
                        cs = slice(c0 * CW, (c0 + 2) * CW)
                        xs = xb[:, MG + c0 * CW : MG + (c0 + 2) * CW]
                        if l == 0:
                            # skip master is uninitialized: write, don't accumulate
                            nc.vector.tensor_scalar_add(Wd(skip_m[:, cs]), Wd(s_ps[:]), bvec(l, 2))
                        else:
                            nc.vector.scalar_tensor_tensor(
                                Wd(skip_m[:, cs]), Wd(s_ps[:]), bvec(l, 2), Wd(skip_m[:, cs]),
                                op0=ALU.add, op1=ALU.add,
                            )
                        if l == layers - 1:
                            stg = workp.tile([C, 2 * CW], F32, tag="stgo", name="stg", bufs=2)
                            nc.vector.scalar_tensor_tensor(
                                Wd(stg[:]), Wd(r_ps[:]), bvec(l, 3), Wd(xs),
                                op0=ALU.add, op1=ALU.add,
                            )
                            nc.sync.dma_start(resid_out[b, :, cs], stg[:])
                            nc.sync.dma_start(skip_out[b, :, cs], skip_m[:, cs])
                        elif (c0 // 2) % 6 == 0:
                            # ACT-assisted residual update on even pairs:
                            # offload PSUM read+bias to ACT, leaving DVE a
                            # cheap fp16 2x-mode add (engine rebalance)
                            r_t = workp.tile([C, 2 * CW], F16, tag="rt", name="r_t", bufs=2)
                            nc.scalar.activation(Wa(r_t[:]), Wa(r_ps[:]),
                                                 AF.Identity, bias=bvec(l, 3))
                            nc.vector.tensor_add(Wd(xs), Wd(xs), Wd(r_t[:]))
                        else:
                            # in-place fp16 residual-master update
                            nc.vector.scalar_tensor_tensor(
                                Wd(xs), Wd(r_ps[:]), bvec(l, 3), Wd(xs),
                                op0=ALU.add, op1=ALU.add,
                            )

                    for c0 in range(0, nch, 2):
                        cc = (c0, c0 + 1)
                        if mm_n == 2 * CW:
                            # single wide tile per pair; one N=1024 matmul per
                            # weight (fewer PE instructions, bufs=1)
                            f_ps = [psump.tile([C, 2 * CW], F32, tag="f", name="f_ps", bufs=1)]
                            g_ps = [psump.tile([C, 2 * CW], F32, tag="g", name="g_ps", bufs=1)]
                            spans = [(0, 2 * CW)]
                        else:
                            # f/g PSUM as 512-wide single-bank tiles, both
                            # chunks of the pair at once (2 bufs per tag)
                            f_ps = [psump.tile([C, CW], F32, tag="f", name="f_ps",
                                               bufs=psum_bufs) for _ in cc]
                            g_ps = [psump.tile([C, CW], F32, tag="g", name="g_ps",
                                               bufs=psum_bufs) for _ in cc]
                            spans = [(0, CW), (CW, CW)]
                        fbp = workp.tile([C, 2 * CW], F16, tag="fb", name="fbp")
                        gbp = workp.tile([C, 2 * CW], F16, tag="gb", name="gbp")
                        # weight-major across the pair: each stationary matrix
                        # streams both chunks back-to-back before switching
                        for j, sh, start in (
                            (0, -s0, True),
                            (1, s1, False),
                            (2, -s0, True),
                            (3, s1, False),
                        ):
                            ps = f_ps if j < 2 else g_ps
                            for i, (off, w) in enumerate(spans):
                                col = c0 * CW + off
                                nc.tensor.matmul(
                                    ps[i][:],
                                    wmat(l, j),
                                    xb[:, MG + col + sh : MG + col + sh + w],
                                    start=start,
                                    stop=not start,
                                )
                            if not start:
                                # both taps done: bias-cast
                                fg_b = fbp if j < 2 else gbp
                                for i, (off, w) in enumerate(spans):
                                    nc.scalar.activation(
                                        Wa(fg_b[:, off:off + w]),
                                        Wa(ps[i][:]),
                                        AF.Identity, bias=bvec(l, j // 2),
                                    )
                        # deferred skip/resid phase: pair k is emitted two
                        # pairs after its in-place xb write became safe, so
                        # the PE never waits on the ACT->GpSimd gating chain
                        if len(pend) >= 2:
                            sr_phase(*pend.pop(0))

                        hp = workp.tile([C, 2 * CW], F16, tag="h", name="hp", bufs=4)
                        if (c0 // 2) % 2 == 1:
                            nc.gpsimd.tensor_mul(Wp(hp[:]), Wp(fbp[:]), Wp(gbp[:]))
                        else:
                            nc.vector.tensor_mul(Wd(hp[:]), Wd(fbp[:]), Wd(gbp[:]))
                        pend.append((c0, hp[:, 0:CW], hp[:, CW:2 * CW], hp))
                    for p in pend:
                        sr_phase(*p)

    nc.finalize()
    return nc


def _sigmoid(x):
    return 1.0 / (1.0 + np.exp(-x))


def _fold(W, G, b, s):
    W = np.asarray(W, np.float32)
    G = np.asarray(G, np.float32)
    b = np.asarray(b, np.float32)
    s = np.asarray(s, np.float32)
    Weff = s[:, :, None, None] * W * _sigmoid(G)
    return Weff.astype(np.float32), (s * b).astype(np.float32)


def _prep_params(Wf, Gf, bf, sf, Wg, Gg, bg, sg, Wr, Gr, br, sr, Ws, Gs, bs, ss,
                 layers=LAYERS):
    Wf_e, bf_e = _fold(Wf, Gf, bf, sf)
    Wg_e, bg_e = _fold(Wg, Gg, bg, sg)
    Wr_e, br_e = _fold(Wr, Gr, br, sr)
    Ws_e, bs_e = _fold(Ws, Gs, bs, ss)

    # wts_host[p, l*6+j, m] = lhsT_j[p, m] = W'_j[m, p] (stationary = W'^T)
    wts_host = np.zeros((C, layers * 6, C), np.float32)
    bias_host = np.zeros((C, layers * 4), np.float32)
    for l in range(layers):
        mats = [Wf_e[l, :, :, 0], Wf_e[l, :, :, 1],
                Wg_e[l, :, :, 0], Wg_e[l, :, :, 1],
                Ws_e[l, :, :, 0], Wr_e[l, :, :, 0]]
        for j, m in enumerate(mats):
            wts_host[:, l * 6 + j, :] = m.T
        bias_host[:, l * 4 + 0] = bf_e[l]
        bias_host[:, l * 4 + 1] = bg_e[l]
        bias_host[:, l * 4 + 2] = bs_e[l]
        bias_host[:, l * 4 + 3] = br_e[l]
    wts_host = wts_host.reshape(C, layers * 6 * C).astype(np.float16)
    return wts_host, bias_host


_NC_CACHE = {}


def _make_runner(nc, n_cores=N_CORES):
    """Persistent jitted multi-core runner (same machinery as the axon path of
    run_bass_kernel_spmd, but reusable across calls without recompiling)."""
    import jax
    from jax.sharding import Mesh, PartitionSpec
    from jax.experimental.shard_map import shard_map
    from concourse.bass2jax import (
        _bass_exec_p, install_neuronx_cc_hook, partition_id_tensor)

    install_neuronx_cc_hook()
    partition_name = nc.partition_id_tensor.name if nc.partition_id_tensor else None
    in_names, out_names, out_avals = [], [], []
    for alloc in nc.m.functions[0].allocations:
        if not isinstance(alloc, mybir.MemoryLocationSet):
            continue
        name = alloc.memorylocations[0].name
        if alloc.kind == "ExternalInput":
            if name != partition_name:
                in_names.append(name)
        elif alloc.kind == "ExternalOutput":
            out_names.append(name)
            out_avals.append(jax.core.ShapedArray(
                tuple(alloc.tensor_shape), mybir.dt.np(alloc.dtype)))
    n_params = len(in_names)
    all_in = list(in_names) + list(out_names)
    if partition_name is not None:
        all_in.append(partition_name)

    def _body(*args):
        operands = list(args)
        if partition_name is not None:
            operands.append(partition_id_tensor())
        outs = _bass_exec_p.bind(
            *operands,
            out_avals=tuple(out_avals), in_names=tuple(all_in),
            out_names=tuple(out_names), lowering_input_output_aliases=(),
            sim_require_finite=True, sim_require_nnan=True, nc=nc)
        return tuple(outs)

    mesh = Mesh(np.asarray(jax.devices()[:n_cores]), ("core",))
    in_specs = (PartitionSpec("core"),) * (n_params + len(out_names))
    out_specs = (PartitionSpec("core"),) * len(out_names)
    fn = jax.jit(shard_map(_body, mesh=mesh, in_specs=in_specs,
                           out_specs=out_specs, check_rep=False),
                 keep_unused=True)
    return fn, in_names, out_names, out_avals


def kernel(x, Wf, Gf, bf, sf, Wg, Gg, bg, sg, Wr, Gr, br, sr, Ws, Gs, bs, ss):
    x = np.asarray(x, np.float32)
    wts_host, bias_host = _prep_params(Wf, Gf, bf, sf, Wg, Gg, bg, sg,
                                       Wr, Gr, br, sr, Ws, Gs, bs, ss)
    if "nc" not in _NC_CACHE:
        _NC_CACHE["nc"] = _build_nc()
    nc = _NC_CACHE["nc"]

    per_core = {
        "x": np.concatenate([x[c * BPC:(c + 1) * BPC] for c in range(N_CORES)], axis=0),
        "wts": np.concatenate([wts_host] * N_CORES, axis=0),
        "biases": np.concatenate([bias_host] * N_CORES, axis=0),
    }
    if "runner" not in _NC_CACHE:
        in_maps = [
            {"x": np.ascontiguousarray(x[c * BPC : (c + 1) * BPC]),
             "wts": wts_host, "biases": bias_host}
            for c in range(N_CORES)
        ]
        res = run_bass_kernel_spmd(nc, in_maps, list(range(N_CORES)))
        resid = np.concatenate([res.results[c]["resid"] for c in range(N_CORES)], axis=0)
        skip = np.concatenate([res.results[c]["skip"] for c in range(N_CORES)], axis=0)
        _NC_CACHE["runner"] = _make_runner(nc)
        return resid, skip

    fn, in_names, out_names, out_avals = _NC_CACHE["runner"]
    args = [per_core[n] for n in in_names]
    zouts = [np.zeros((N_CORES * av.shape[0], *av.shape[1:]), av.dtype)
             for av in out_avals]
    outs = fn(*args, *zouts)
    res = {n: np.asarray(outs[i]) for i, n in enumerate(out_names)}
    resid = res["resid"].reshape(B, C, L)
    skip = res["skip"].reshape(B, C, L)
    return resid, skip


# revision 23
# speedup vs baseline: 1.1978x; 1.1978x over previous
"""Trainium2 Bass kernel for nn_BitwiseWavenetBlock (10-layer gated WaveNet block).

Strategy: data-parallel over batch (16 batches -> 8 cores x 2 sequential).
Each core runs the full 10-layer block on [128, 16384] activations resident in
SBUF. Dilated k=2 convs are pairs of PSUM-accumulated 128x128 matmuls against
shifted windows of an fp16 activation buffer with zeroed margins. Weight
gating (W * sigmoid(G)), per-channel scale s and biases are folded on the host.

Engine split per 1024-col chunk pair: PE 12 matmuls (N=512, weight-major so
each stationary matrix streams both chunks); ACT bias-add + fp16 cast of the
filter/gate PSUM halves (512-wide, double-buffered banks); DVE gating multiply
plus fused (psum + bias) + master accumulations (1024-wide); on alternating
pairs the residual update is ACT-assisted to balance ACT/DVE. The residual
master lives in fp16 directly in the conv input buffer xb (updated in place);
the skip master accumulates in fp32. Each pair's skip/residual 1x1 matmuls and
master updates are deferred two pairs so the in-place xb write never races the
backward conv tap and the PE never waits on the gating chain.
"""

import sys

sys.path.insert(0, "/opt/trn_rl_repo")

import numpy as np

import concourse.bass as bass
import concourse.bacc as bacc
import concourse.mybir as mybir
import concourse.tile as tile
from concourse.bass_utils import run_bass_kernel_spmd

F32 = mybir.dt.float32
F16 = mybir.dt.float16
AF = mybir.ActivationFunctionType
ALU = mybir.AluOpType

N_CORES = 8
LAYERS = 10
C = 128          # channels (= partitions)
L = 16384        # sequence length
B = 16           # total batch
BPC = B // N_CORES  # batches per core (sequential)
CW = 512         # chunk width (1 PSUM bank)
MG = 256         # xb margin (>= max shift 2^8)
NH = CW // 512   # matmul halves per chunk


def _build_nc(bpc=BPC, l_len=L, layers=LAYERS, psum_bufs=2, reps=1, ew_width=None,
              mm_n=512, ew_act=None, ew_dve=None, ew_pool=None):
    if mm_n > 512:
        bass.BassTensorEngine.MAX_MOVING_FREE_DIM_SIZE = mm_n
    # ew_*: attribution-only knobs - slice that engine's ops to this many
    # columns (keeps op count/deps, removes streaming work). None = full.
    ew_act = ew_act if ew_act is not None else ew_width
    ew_dve = ew_dve if ew_dve is not None else ew_width
    ew_pool = ew_pool if ew_pool is not None else ew_width
    def _w(ap, w):
        return ap if w is None else ap[:, :w]
    def Wa(ap):
        return _w(ap, ew_act)
    def Wd(ap):
        return _w(ap, ew_dve)
    def Wp(ap):
        return _w(ap, ew_pool)
    nch = l_len // CW
    nc = bacc.Bacc(None)
    x_in = nc.declare_dram_parameter("x", [bpc, C, l_len], F32, isOutput=False)
    wts_in = nc.declare_dram_parameter("wts", [C, layers * 6 * C], F16, isOutput=False)
    bias_in = nc.declare_dram_parameter("biases", [C, layers * 4], F32, isOutput=False)
    resid_out = nc.declare_dram_parameter("resid", [bpc, C, l_len], F32, isOutput=True)
    skip_out = nc.declare_dram_parameter("skip", [bpc, C, l_len], F32, isOutput=True)

    with tile.TileContext(nc) as tc:
        with (
            tc.tile_pool(name="constp", bufs=1) as constp,
            tc.tile_pool(name="masterp", bufs=1) as masterp,
            tc.tile_pool(name="workp", bufs=4) as workp,
            tc.tile_pool(name="psump", bufs=1, space="PSUM") as psump,
        ):
            wts = constp.tile([C, layers * 6 * C], F16)
            nc.sync.dma_start(wts[:], wts_in[:])
            biases = constp.tile([C, layers * 4], F32)
            nc.sync.dma_start(biases[:], bias_in[:])

            skip_m = masterp.tile([C, l_len], F32)
            xb = masterp.tile([C, l_len + 2 * MG], F16)
            # zero the conv margins; extend 512 cols into the data region
            # (overwritten by the input casts) so subtile dep-tracking can't
            # miss the narrow margin overlap of boundary conv taps
            nc.vector.memset(xb[:, 0 : MG + 512], 0.0)
            nc.vector.memset(xb[:, MG + l_len - 512 : l_len + 2 * MG], 0.0)

            def wmat(l, j):
                return wts[:, (l * 6 + j) * C : (l * 6 + j + 1) * C]

            def bvec(l, j):
                return biases[:, l * 4 + j : l * 4 + j + 1]

            def xbc(c):
                return xb[:, MG + c * CW : MG + (c + 1) * CW]

            for b in range(bpc * reps):
                b = b % bpc
                for c in range(nch):
                    stg_in = workp.tile([C, CW], F32, tag="stg", name="stg_in", bufs=8)
                    nc.sync.dma_start(stg_in[:], x_in[b, :, c * CW : (c + 1) * CW])
                    nc.vector.tensor_copy(Wd(xbc(c)), Wd(stg_in[:]))
                for l in range(layers):
                    s0 = 1 if l == 0 else 2 ** (l - 1)
                    s1 = 0 if l == 0 else 2 ** (l - 1)
                    h_pair = []
                    pend = []

                    def sr_phase(c0, h_a, h_b, h_full, l=l, b=b):
                        # skip/resid 1x1 convs + master updates over a PAIR of
                        # chunks: 1024-wide PSUM tiles keep the DVE ops wide
                        s_ps = psump.tile([C, 2 * CW], F32, tag="s", name="s_ps", bufs=1)
                        r_ps = psump.tile([C, 2 * CW], F32, tag="r", name="r_ps", bufs=1)
                        if mm_n == 2 * CW:
                            nc.tensor.matmul(s_ps[:], wmat(l, 4), h_full[:], start=True, stop=True)
                            nc.tensor.matmul(r_ps[:], wmat(l, 5), h_full[:], start=True, stop=True)
                        else:
                            for i, h_t in ((0, h_a), (1, h_b)):
                                hs = slice(i * CW, (i + 1) * CW)
                                nc.tensor.matmul(s_ps[:, hs], wmat(l, 4), h_t[:], start=True, stop=True)
                            for i, h_t in ((0, h_a), (1, h_b)):
                                hs = slice(i * CW, (i + 1) * CW)
                                nc.tensor.matmul(r_ps[:, hs], wmat(l, 5), h_t[:], start=True, stop=True)
                        cs = slice(c0 * CW, (c0 + 2) * CW)
                        xs = xb[:, MG + c0 * CW : MG + (c0 + 2) * CW]
                        if l == 0:
                            # skip master is uninitialized: write, don't accumulate
                            nc.vector.tensor_scalar_add(Wd(skip_m[:, cs]), Wd(s_ps[:]), bvec(l, 2))
                        else:
                            nc.vector.scalar_tensor_tensor(
                                Wd(skip_m[:, cs]), Wd(s_ps[:]), bvec(l, 2), Wd(skip_m[:, cs]),
                                op0=ALU.add, op1=ALU.add,
                            )
                        if l == layers - 1:
                            stg = workp.tile([C, 2 * CW], F32, tag="stgo", name="stg", bufs=2)
                            nc.vector.scalar_tensor_tensor(
                                Wd(stg[:]), Wd(r_ps[:]), bvec(l, 3), Wd(xs),
                                op0=ALU.add, op1=ALU.add,
                            )
                            nc.sync.dma_start(resid_out[b, :, cs], stg[:])
                            nc.sync.dma_start(skip_out[b, :, cs], skip_m[:, cs])
                        elif (c0 // 2) % 6 == 0:
                            # ACT-assisted residual update on even pairs:
                            # offload PSUM read+bias to ACT, leaving DVE a
                            # cheap fp16 2x-mode add (engine rebalance)
                            r_t = workp.tile([C, 2 * CW], F16, tag="rt", name="r_t", bufs=2)
                            nc.scalar.activation(Wa(r_t[:]), Wa(r_ps[:]),
                                                 AF.Identity, bias=bvec(l, 3))
                            nc.vector.tensor_add(Wd(xs), Wd(xs), Wd(r_t[:]))
                        else:
                            # in-place fp16 residual-master update
                            nc.vector.scalar_tensor_tensor(
                                Wd(xs), Wd(r_ps[:]), bvec(l, 3), Wd(xs),
                                op0=ALU.add, op1=ALU.add,
                            )

                    for c0 in range(0, nch, 2):
                        cc = (c0, c0 + 1)
                        if mm_n == 2 * CW:
                            # single wide tile per pair; one N=1024 matmul per
                            # weight (fewer PE instructions, bufs=1)
                            f_ps = [psump.tile([C, 2 * CW], F32, tag="f", name="f_ps", bufs=1)]
                            g_ps = [psump.tile([C, 2 * CW], F32, tag="g", name="g_ps", bufs=1)]
                            spans = [(0, 2 * CW)]
                        else:
                            # f/g PSUM as 512-wide single-bank tiles, both
                            # chunks of the pair at once (2 bufs per tag)
                            f_ps = [psump.tile([C, CW], F32, tag="f", name="f_ps",
                                               bufs=psum_bufs) for _ in cc]
                            g_ps = [psump.tile([C, CW], F32, tag="g", name="g_ps",
                                               bufs=psum_bufs) for _ in cc]
                            spans = [(0, CW), (CW, CW)]
                        fbp = workp.tile([C, 2 * CW], F16, tag="fb", name="fbp")
                        gbp = workp.tile([C, 2 * CW], F16, tag="gb", name="gbp")
                        # weight-major across the pair: each stationary matrix
                        # streams both chunks back-to-back before switching
                        for j, sh, start in (
                            (0, -s0, True),
                            (1, s1, False),
                            (2, -s0, True),
                            (3, s1, False),
                        ):
                            ps = f_ps if j < 2 else g_ps
                            for i, (off, w) in enumerate(spans):
                                col = c0 * CW + off
                                nc.tensor.matmul(
                                    ps[i][:],
                                    wmat(l, j),
                                    xb[:, MG + col + sh : MG + col + sh + w],
                                    start=start,
                                    stop=not start,
                                )
                            if not start:
                                # both taps done: bias-cast
                                fg_b = fbp if j < 2 else gbp
                                for i, (off, w) in enumerate(spans):
                                    nc.scalar.activation(
                                        Wa(fg_b[:, off:off + w]),
                                        Wa(ps[i][:]),
                                        AF.Identity, bias=bvec(l, j // 2),
                                    )
                        # deferred skip/resid phase: pair k is emitted two
                        # pairs after its in-place xb write became safe, so
                        # the PE never waits on the ACT->GpSimd gating chain
                        if len(pend) >= 2:
                            sr_phase(*pend.pop(0))

                        hp = workp.tile([C, 2 * CW], F16, tag="h", name="hp", bufs=4)
                        if (c0 // 2) % 2 == 1:
                            nc.gpsimd.tensor_mul(Wp(hp[:]), Wp(fbp[:]), Wp(gbp[:]))
                        else:
                            nc.vector.tensor_mul(Wd(hp[:]), Wd(fbp[:]), Wd(gbp[:]))
                        pend.append((c0, hp[:, 0:CW], hp[:, CW:2 * CW], hp))
                    for p in pend:
                        sr_phase(*p)

    nc.finalize()
    return nc


def _sigmoid(x):
    return 1.0 / (1.0 + np.exp(-x))


def _fold(W, G, b, s):
    W = np.asarray(W, np.float32)
    G = np.asarray(G, np.float32)
    b = np.asarray(b, np.float32)
    s = np.asarray(s, np.float32)
    Weff = s[:, :, None, None] * W * _sigmoid(G)
    return Weff.astype(np.float32), (s * b).astype(np.float32)


def _prep_params(Wf, Gf, bf, sf, Wg, Gg, bg, sg, Wr, Gr, br, sr, Ws, Gs, bs, ss,
                 layers=LAYERS):
    Wf_e, bf_e = _fold(Wf, Gf, bf, sf)
    Wg_e, bg_e = _fold(Wg, Gg, bg, sg)
    Wr_e, br_e = _fold(Wr, Gr, br, sr)
    Ws_e, bs_e = _fold(Ws, Gs, bs, ss)

    # wts_host[p, l*6+j, m] = lhsT_j[p, m] = W'_j[m, p] (stationary = W'^T)
    wts_host = np.zeros((C, layers * 6, C), np.float32)
    bias_host = np.zeros((C, layers * 4), np.float32)
    for l in range(layers):
        mats = [Wf_e[l, :, :, 0], Wf_e[l, :, :, 1],
                Wg_e[l, :, :, 0], Wg_e[l, :, :, 1],
                Ws_e[l, :, :, 0], Wr_e[l, :, :, 0]]
        for j, m in enumerate(mats):
            wts_host[:, l * 6 + j, :] = m.T
        bias_host[:, l * 4 + 0] = bf_e[l]
        bias_host[:, l * 4 + 1] = bg_e[l]
        bias_host[:, l * 4 + 2] = bs_e[l]
        bias_host[:, l * 4 + 3] = br_e[l]
    wts_host = wts_host.reshape(C, layers * 6 * C).astype(np.float16)
    return wts_host, bias_host


_NC_CACHE = {}


def _make_runner(nc, n_cores=N_CORES):
    """Persistent jitted multi-core runner (same machinery as the axon path of
    run_bass_kernel_spmd, but reusable across calls without recompiling)."""
    import jax
    from jax.sharding import Mesh, PartitionSpec
    from jax.experimental.shard_map import shard_map
    from concourse.bass2jax import (
        _bass_exec_p, install_neuronx_cc_hook, partition_id_tensor)

    install_neuronx_cc_hook()
    partition_name = nc.partition_id_tensor.name if nc.partition_id_tensor else None
    in_names, out_names, out_avals = [], [], []
    for alloc in nc.m.functions[0].allocations:
        if not isinstance(alloc, mybir.MemoryLocationSet):
            continue
        name = alloc.memorylocations[0].name
        if alloc.kind == "ExternalInput":
            if name != partition_name:
                in_names.append(name)
        elif alloc.kind == "ExternalOutput":
            out_names.append(name)
            out_avals.append(jax.core.ShapedArray(
                tuple(alloc.tensor_shape), mybir.dt.np(alloc.dtype)))
    n_params = len(in_names)
    all_in = list(in_names) + list(out_names)
    if partition_name is not None:
        all_in.append(partition_name)

    def _body(*args):
        operands = list(args)
        if partition_name is not None:
            operands.append(partition_id_tensor())
        outs = _bass_exec_p.bind(
            *operands,
            out_avals=tuple(out_avals), in_names=tuple(all_in),
            out_names=tuple(out_names), lowering_input_output_aliases=(),
            sim_require_finite=True, sim_require_nnan=True, nc=nc)
        return tuple(outs)

    mesh = Mesh(np.asarray(jax.devices()[:n_cores]), ("core",))
    in_specs = (PartitionSpec("core"),) * (n_params + len(out_names))
    out_specs = (PartitionSpec("core"),) * len(out_names)
    fn = jax.jit(shard_map(_body, mesh=mesh, in_specs=in_specs,
                           out_specs=out_specs, check_rep=False),
                 keep_unused=True)
    return fn, in_names, out_names, out_avals


def kernel(x, Wf, Gf, bf, sf, Wg, Gg, bg, sg, Wr, Gr, br, sr, Ws, Gs, bs, ss):
    x = np.asarray(x, np.float32)
    wts_host, bias_host = _prep_params(Wf, Gf, bf, sf, Wg, Gg, bg, sg,
                                       Wr, Gr, br, sr, Ws, Gs, bs, ss)
    if "nc" not in _NC_CACHE:
        _NC_CACHE["nc"] = _build_nc()
    nc = _NC_CACHE["nc"]

    per_core = {
        "x": np.concatenate([x[c * BPC:(c + 1) * BPC] for c in range(N_CORES)], axis=0),
        "wts": np.concatenate([wts_host] * N_CORES, axis=0),
        "biases": np.concatenate([bias_host] * N_CORES, axis=0),
    }
    if "runner" not in _NC_CACHE:
        in_maps = [
            {"x": np.ascontiguousarray(x[c * BPC : (c + 1) * BPC]),
             "wts": wts_host, "biases": bias_host}
            for c in range(N_CORES)
        ]
        res = run_bass_kernel_spmd(nc, in_maps, list(range(N_CORES)))
        resid = np.concatenate([res.results[c]["resid"] for c in range(N_CORES)], axis=0)
        skip = np.concatenate([res.results[c]["skip"] for c in range(N_CORES)], axis=0)
        _NC_CACHE["runner"] = _make_runner(nc)
        return resid, skip

    fn, in_names, out_names, out_avals = _NC_CACHE["runner"]
    args = [per_core[n] for n in in_names]
    zouts = [np.zeros((N_CORES * av.shape[0], *av.shape[1:]), av.dtype)
             for av in out_avals]
    outs = fn(*args, *zouts)
    res = {n: np.asarray(outs[i]) for i, n in enumerate(out_names)}
    resid = res["resid"].reshape(B, C, L)
    skip = res["skip"].reshape(B, C, L)
    return resid, skip
